# revision 9
# baseline (speedup 1.0000x reference)
"""nn_CausalSelfAttention3 — Trainium2 Bass kernel, full-input contract.

Sharding: pure data-parallel over batch B=4096 -> 8 cores x 512 samples.

Math (hand-derived from the reference block-merge attention):
  - y0 (summary queries) never affects the output (dropped by un-merge).
  - group 0 tokens: plain causal attention within the 16-token group.
  - group g>=1 token p attends to summary key y1[b,:,g-1] (value y2) plus
    tokens 0..p of its own group.

All PE matmuls use full K=128 contraction (partition-strip matmuls and
gpsimd compute are broken on this stack). Per-head score separation is
achieved by folding W into per-head 128x128 operators on the host:
  S_h^T[k,q] = x_k^T (Wk_h Wq_h^T) x_q  ->  uT_h = A_h^T-stationary @ x^T,
  then S = xt_t-stationary @ uT_h.
Summary attention: skT2 = Wq_h @ y1-key embedded in a block-diagonal
[128, 128] host tensor (col j = h*32 + t*8 + s2*3 + g), summary values in
sv_wide [128 j, 128 ch] (head-block-diagonal) so AV/den contractions stay
K=128 with mask-enforced zeros.

Per super-tile (512 tokens = 8 samples): u matmuls -> evac bf16; v = x@Wv;
summary scores -> exp (ScalarE) -> mask (VectorE); per 128-token tile:
4 S matmuls (shared stationary) -> exp -> mask -> AV + denominators in one
PSUM tile -> reciprocal + per-head scale evac (split ACT/DVE) -> PE
transpose (host identity) -> proj -> DMA out.
"""

import os
import sys

sys.path.insert(0, "/opt/trn_rl_repo")

import numpy as np
import ml_dtypes

import concourse.bass as bass
import concourse.bacc as bacc_mod
import concourse.mybir as mybir
import concourse.tile as tile
from concourse.bass_utils import run_bass_kernel_spmd

BF16 = mybir.dt.bfloat16
F32 = mybir.dt.float32
NPBF16 = ml_dtypes.bfloat16

B, T, DIM = 4096, 64, 128
H, HS, G, GT = 4, 32, 4, 16
N_CORES = 8
B_SH = B // N_CORES          # 512 samples per core
SUP = 512                    # tokens per super-tile (8 samples)
NSUP = B_SH * T // SUP       # 64 super-tiles per core
SCALE = float(1.0 / np.sqrt(HS))

# w columns: A^T x4 | Wv | Wp | ident | ones | H_ind
WC_A, WC_WV, WC_WP, WC_ID, WC_ONE, WC_HI = 0, 512, 640, 768, 896, 897
WCOLS = 901

LAST_RESULTS = None          # BassKernelResults stash for test.py


def build_nc(nsup=NSUP):
    nc = bacc_mod.Bacc()
    xt = nc.declare_dram_parameter("xt", [nsup, 128, SUP], BF16, isOutput=False)
    st = nc.declare_dram_parameter("st", [nsup, 128, 256], BF16, isOutput=False)
    w = nc.declare_dram_parameter("w", [128, WCOLS], BF16, isOutput=False)
    mk = nc.declare_dram_parameter("mk", [128, 1024], BF16, isOutput=False)
    y = nc.declare_dram_parameter("y", [nsup, SUP, DIM], F32, isOutput=True)

    with tile.TileContext(nc) as tc:
        from contextlib import ExitStack
        with ExitStack() as ctx:
            const = ctx.enter_context(tc.tile_pool(name="const", bufs=1))
            sb = ctx.enter_context(tc.tile_pool(name="sb", bufs=3))
            sbE = ctx.enter_context(tc.tile_pool(name="sbE", bufs=3))
            sb2 = ctx.enter_context(tc.tile_pool(name="sb2", bufs=2))
            psU = ctx.enter_context(tc.tile_pool(name="psU", bufs=1, space="PSUM"))
            psS = ctx.enter_context(tc.tile_pool(name="psS", bufs=1, space="PSUM"))
            psOD = ctx.enter_context(tc.tile_pool(name="psOD", bufs=2, space="PSUM"))
            psT = ctx.enter_context(tc.tile_pool(name="psT", bufs=1, space="PSUM"))

            w_sb = const.tile([128, WCOLS], BF16)
            nc.sync.dma_start(w_sb[:], w[:])
            mk_sb = const.tile([128, 1024], BF16)
            nc.sync.dma_start(mk_sb[:], mk[:])
            AT = [w_sb[:, WC_A + i * 128:WC_A + (i + 1) * 128] for i in range(4)]
            Wv = w_sb[:, WC_WV:WC_WV + 128]
            Wp = w_sb[:, WC_WP:WC_WP + 128]
            ident = w_sb[:, WC_ID:WC_ID + 128]
            ones = w_sb[:, WC_ONE:WC_ONE + 1]
            H_ind = w_sb[:, WC_HI:WC_HI + 4]
            maskm = mk_sb[:, 0:512]
            summask = mk_sb[:, 512:1024]

            for s in range(nsup):
                xt_sb = sb.tile([128, SUP], BF16, tag="xt")
                nc.gpsimd.dma_start(xt_sb[:], xt[s])
                st_sb = sb.tile([128, 256], BF16, tag="st")
                nc.gpsimd.dma_start(st_sb[:], st[s])
                skT2 = st_sb[:, 0:128]
                sv_wide = st_sb[:, 128:256]

                # uT_h = A_h @ x^T, pairs of heads -> [128, 1024] psum
                u_sb = sb2.tile([128, 2048], BF16, tag="u")
                for p in range(2):
                    u_ps = psU.tile([128, 1024], F32, tag="u")
                    nc.tensor.matmul(u_ps[:, 0:512], AT[2 * p], xt_sb[:],
                                     start=True, stop=True)
                    nc.tensor.matmul(u_ps[:, 512:1024], AT[2 * p + 1], xt_sb[:],
                                     start=True, stop=True)
                    if p == 0:
                        nc.scalar.copy(u_sb[:, 0:1024], u_ps[:])
                    else:
                        nc.vector.tensor_copy(u_sb[:, 1024:2048], u_ps[:])

                # v = x @ Wv   [tok, ch] per 128-token tile, packed [128, 512]
                v_ps = psU.tile([128, 1024], F32, tag="u")
                for t in range(4):
                    nc.tensor.matmul(v_ps[:, t * 128:(t + 1) * 128],
                                     xt_sb[:, t * 128:(t + 1) * 128], Wv,
                                     start=True, stop=True)
                v = sb.tile([128, SUP], BF16, tag="vb")
                nc.vector.tensor_copy(v[:], v_ps[:, 0:512])

                # summary scores: [128 j, 512 q] = skT2.T @ x^T
                ss_ps = psU.tile([128, 1024], F32, tag="u")
                nc.tensor.matmul(ss_ps[:, 0:512], skT2, xt_sb[:],
                                 start=True, stop=True)
                es_raw = sbE.tile([128, SUP], BF16, tag="esr")
                nc.scalar.activation(es_raw[:], ss_ps[:, 0:512],
                                     mybir.ActivationFunctionType.Exp, scale=SCALE)
                es = sbE.tile([128, SUP], BF16, tag="es")
                nc.vector.tensor_tensor(es[:], es_raw[:], summask,
                                        mybir.AluOpType.mult)

                oT_ps = psT.tile([128, SUP], BF16, tag="oT")
                for tp in range(2):
                    # S for two token-tiles in one [128, 1024] psum
                    s_ps = psS.tile([128, 1024], F32, tag="S")
                    for ti in range(2):
                        t = 2 * tp + ti
                        tc0 = t * 128
                        for h in range(H):
                            nc.tensor.matmul(
                                s_ps[:, ti * 512 + h * 128:ti * 512 + (h + 1) * 128],
                                xt_sb[:, tc0:tc0 + 128],
                                u_sb[:, h * 512 + tc0:h * 512 + tc0 + 128],
                                start=True, stop=True)
                    e_raw = sbE.tile([128, 1024], BF16, tag="er")
                    nc.scalar.activation(e_raw[:], s_ps[:],
                                         mybir.ActivationFunctionType.Exp,
                                         scale=SCALE)

                    for ti in range(2):
                        t = 2 * tp + ti
                        tc0, tc1 = t * 128, (t + 1) * 128
                        e = sbE.tile([128, SUP], BF16, tag="e")
                        nc.vector.tensor_tensor(e[:], e_raw[:, ti * 512:(ti + 1) * 512],
                                                maskm, mybir.AluOpType.mult)

                        # o (cols 0:128) + den (cols 128:132); single psum group
                        od = psOD.tile([128, 132], F32, tag="od")
                        est = es[:, tc0:tc1]
                        nc.tensor.matmul(od[:, 0:128], est, sv_wide,
                                         start=True, stop=False)
                        nc.tensor.matmul(od[:, 128:132], est, H_ind,
                                         start=False, stop=False)
                        for h in range(H):
                            hp = h * HS
                            eh = e[:, h * 128:(h + 1) * 128]
                            nc.tensor.matmul(od[:, hp:hp + HS], eh,
                                             v[:, tc0 + hp:tc0 + hp + HS],
                                             start=False, stop=False)
                            nc.tensor.matmul(od[:, 128 + h:129 + h], eh, ones,
                                             start=False, stop=(h == H - 1))

                        recip = sb.tile([128, 4], F32, tag="recip")
                        nc.vector.reciprocal(recip[:], od[:, 128:132])
                        o_bf = sb.tile([128, 128], BF16, tag="ob")
                        for h in range(H):
                            hp = h * HS
                            if h < 2:
                                nc.scalar.mul(o_bf[:, hp:hp + HS],
                                              od[:, hp:hp + HS],
                                              recip[:, h:h + 1])
                            else:
                                nc.vector.tensor_scalar_mul(
                                    o_bf[:, hp:hp + HS], od[:, hp:hp + HS],
                                    recip[:, h:h + 1])

                        nc.tensor.transpose(oT_ps[:, tc0:tc1], o_bf[:], ident)

                oT = sb2.tile([128, SUP], BF16, tag="oTb")
                nc.scalar.copy(oT[:], oT_ps[:])
                y_ps = psT.tile([128, SUP], F32, tag="y")
                for t in range(4):
                    nc.tensor.matmul(y_ps[:, t * 128:(t + 1) * 128],
                                     oT[:, t * 128:(t + 1) * 128], Wp,
                                     start=True, stop=True)
                y_sb = sb2.tile([128, SUP], F32, tag="ysb")
                nc.vector.tensor_copy(y_sb[:], y_ps[:])
                for t in range(4):
                    nc.sync.dma_start(y[s, t * 128:(t + 1) * 128, :],
                                      y_sb[:, t * 128:(t + 1) * 128])
    nc.compile()
    return nc


def host_prep(x, y1, y2, W_attn, W_proj):
    """Build per-core staged inputs. Returns list of in_maps."""
    Wf = np.asarray(W_attn, np.float32)
    Wpf = np.asarray(W_proj, np.float32)

    # xt: [cores, NSUP, 128 dim, 512 tok]
    xb = np.asarray(x, np.float32).astype(NPBF16)
    xt = np.ascontiguousarray(
        xb.reshape(N_CORES, NSUP, SUP, DIM).swapaxes(2, 3))

    # st: skT2 | sv_wide  -> [cores, NSUP, 128, 256]
    st = np.zeros((N_CORES, NSUP, 128, 256), np.float32)
    y1r = np.asarray(y1, np.float32)[:, :, :3, 0, :].reshape(
        N_CORES, NSUP, 4, 2, H, 3, HS)      # [c,s,t,s2,h,g,ch]
    y2r = np.asarray(y2, np.float32)[:, :, :3, 0, :].reshape(
        N_CORES, NSUP, 4, 2, H, 3, HS)
    for h in range(H):
        Wq_h = Wf[:, h * HS:(h + 1) * HS]                    # [128 dim, 32]
        # skT2 col j = h*32 + t*8 + s2*3 + g  (cols h*32+t*8+6..7 stay 0)
        sk = np.einsum("dc,nstpgc->nstpgd", Wq_h, y1r[:, :, :, :, h])
        sk = sk.reshape(N_CORES, NSUP, 4, 6, 128)            # [c,s,t,(s2 g),dim]
        for t in range(4):
            j0 = h * HS + t * 8
            st[:, :, :, j0:j0 + 6] = sk[:, :, t].transpose(0, 1, 3, 2)
            # sv_wide rows j, head-block cols
            st[:, :, j0:j0 + 6, 128 + h * HS:128 + (h + 1) * HS] = \
                y2r[:, :, t, :, h].reshape(N_CORES, NSUP, 6, HS)
    st = st.astype(NPBF16)

    # w: A^T x4 | Wv | Wp | ident | ones | H_ind
    w = np.zeros((128, WCOLS), np.float32)
    for h in range(H):
        Wq_h = Wf[:, h * HS:(h + 1) * HS]
        Wk_h = Wf[:, DIM + h * HS:DIM + (h + 1) * HS]
        w[:, WC_A + h * 128:WC_A + (h + 1) * 128] = Wq_h @ Wk_h.T  # A_h^T
    w[:, WC_WV:WC_WV + 128] = Wf[:, 2 * DIM:3 * DIM]
    w[:, WC_WP:WC_WP + 128] = Wpf
    w[:, WC_ID:WC_ID + 128] = np.eye(128)
    w[:, WC_ONE] = 1.0
    ji = np.arange(128)
    for hp in range(4):
        w[ji[ji // 32 == hp], WC_HI + hp] = 1.0
    w = w.astype(NPBF16)

    # masks
    qi = np.arange(128)
    mask_t = (((qi[:, None] // GT) == (qi[None, :] // GT)) &
              ((qi[:, None] % GT) <= (qi[None, :] % GT))).astype(np.float32)
    mk = np.zeros((128, 1024), np.float32)
    mk[:, 0:512] = np.tile(mask_t, (1, H))
    q5 = np.arange(SUP)
    slot_q, grp_q = q5 // T, (q5 % T) // GT
    for h in range(H):
        for t in range(4):
            for s2 in range(2):
                for g in range(3):
                    j = h * HS + t * 8 + s2 * 3 + g
                    mk[j, 512:1024] = ((slot_q == 2 * t + s2) &
                                       (grp_q == g + 1)).astype(np.float32)
    mk = mk.astype(NPBF16)

    return [{"xt": xt[i], "st": st[i], "w": w, "mk": mk}
            for i in range(N_CORES)]


def kernel(x, y0, y1, y2, W_attn, W_proj):
    global LAST_RESULTS
    in_maps = host_prep(x, y1, y2, W_attn, W_proj)
    nc = build_nc()
    res = run_bass_kernel_spmd(nc, in_maps, list(range(N_CORES)))
    LAST_RESULTS = res
    out = np.concatenate(
        [r["y"].reshape(B_SH, T, DIM) for r in res.results], axis=0)
    return np.ascontiguousarray(out.astype(np.float32))
